# revision 1
# baseline (speedup 1.0000x reference)
"""Trainium2 Bass kernel for a dense transformer block (tensor-parallel on 8 cores).

Reference computation (fp32, B=1, T=2048, C=4096, 32 heads / 8 KV groups GQA,
full-head RoPE, learned per-(token,head) score scaling, SwiGLU MLP FFN=11008):

    n1 = rmsnorm(x, w1)
    h  = attn(n1) @ proj_w.T          (per-head scores scaled by
                                       mean(relu(n1 @ scale_w.T + scale_b)))
    x1 = x + h
    out = x1 + mlp(rmsnorm(x1, w2))

Sharding: core g owns KV group g = 4 q heads + 1 k + 1 v (attn_w rows
g*768:(g+1)*768), the matching scale_w/proj_w slices, and FFN rows
g*1376:(g+1)*1376 (zero-padded to 1408 = 11*128).  Norm weights are folded
into the following matmul weights on the host.  Everything on device lives
transposed (features on partitions, tokens on the free axis) so no on-device
transposes are needed anywhere; V is additionally produced token-major by a
second matmul orientation so softmax(S) @ V needs no transpose either.

Collectives: attention proj partials AllReduce in bf16; the MLP output is
ReduceScattered in fp32 with x1/8 pre-folded into every core's contribution,
so the scatter output IS the final output slice (rows g*512:(g+1)*512 of
out.T) with no core-dependent addressing in the SPMD program.
"""

import numpy as np
import ml_dtypes

import concourse.mybir as mybir
import concourse.tile as tile
from concourse import bacc
from concourse.bass_utils import run_bass_kernel_spmd

BF = ml_dtypes.bfloat16
F32 = mybir.dt.float32
BF16 = mybir.dt.bfloat16
AF = mybir.ActivationFunctionType
ADD = mybir.AluOpType.add

N_CORES = 8
T = 2048
C = 4096
HS = 128
NH = 4            # q heads per core
CT = C // 128     # 32 c-tiles
TCH = 512         # token chunk, phases A-C
NCH = T // TCH
KT = T // 128     # 16 key tiles
FT = 11           # ffn tiles per core (1376 padded to 1408)
TM = 1024         # token half, phases D-E
EPS = 1e-5
SF = 1.0 / float(np.sqrt(HS))

_CACHE = {}


def _build(cc=True):
    # cc=False replaces collectives with direct buffer reads (wrong results,
    # same dataflow) so the kernel can run in the local TimelineSim
    nc = bacc.Bacc(None, target_bir_lowering=False, num_devices=N_CORES)

    xT = nc.dram_tensor("xT", [C, T], F32, kind="ExternalInput")
    cosT = nc.dram_tensor("cosT", [128, T], F32, kind="ExternalInput")
    sinT = nc.dram_tensor("sinT", [128, T], F32, kind="ExternalInput")
    # m-tiles 0-3: q heads, 4: k, 5: v, 6-9: scale heads
    wA = nc.dram_tensor("wA", [10, 128, C], BF16, kind="ExternalInput")
    scale_b = nc.dram_tensor("scale_b", [128, NH], F32, kind="ExternalInput")
    proj_w = nc.dram_tensor("proj_w", [128, NH, C], BF16, kind="ExternalInput")
    gate_w = nc.dram_tensor("gate_w", [FT, 128, C], BF16, kind="ExternalInput")
    up_w = nc.dram_tensor("up_w", [FT, 128, C], BF16, kind="ExternalInput")
    down_w = nc.dram_tensor("down_w", [CT, 128, FT * 128], BF16, kind="ExternalInput")
    outT = nc.dram_tensor("outT", [512, T], F32, kind="ExternalOutput")

    with tile.TileContext(nc) as tc:
        with (
            tc.tile_pool(name="persist", bufs=1) as pp,
            tc.tile_pool(name="dram", bufs=1, space="DRAM") as dram,
        ):
            inv_c = pp.tile([128, 128], BF16, name="inv_c")
            nc.vector.memset(inv_c[:], 1.0 / C)
            mean_sf = pp.tile([128, 128], BF16, name="mean_sf")
            nc.vector.memset(mean_sf[:], SF / HS)
            ones128 = pp.tile([128, 128], BF16, name="ones128")
            nc.vector.memset(ones128[:], 1.0)
            eps_sb = pp.tile([128, 1], F32, name="eps_sb")
            nc.vector.memset(eps_sb[:], EPS)

            # per-token-chunk collective buffers so each collective can fire
            # as soon as its chunk's partials are written
            cc1_in = [dram.tile([C, TCH], BF16, name=f"cc1_in{i}") for i in range(NCH)]
            cc1_out = [
                dram.tile([C, TCH], BF16, name=f"cc1_out{i}", addr_space="Shared")
                for i in range(NCH)
            ]
            cc2_in = [dram.tile([C, TCH], F32, name=f"cc2_in{i}") for i in range(NCH)]
            cc2_out = [dram.tile([512, TCH], F32, name=f"cc2_out{i}") for i in range(NCH)]

            with tc.tile_pool(name="pqkv", bufs=1) as pqkv:
                Q_sb = pqkv.tile([128, NH, T], BF16, name="Q_sb")
                K_sb = pqkv.tile([128, T], BF16, name="K_sb")
                V_sb = pqkv.tile([128, KT, 128], BF16, name="V_sb")
                yT_sb = pqkv.tile([128, NH, T], BF16, name="yT_sb")

                # ---------- Phase A: rmsnorm1, score-scale, qkv, rope ------
                with (
                    tc.tile_pool(name="pa", bufs=1) as pa,
                    tc.tile_pool(name="pa2", bufs=2) as pa2,
                    tc.tile_pool(name="pa3", bufs=3) as pa3,
                    tc.tile_pool(name="psA", bufs=1, space="PSUM") as psA,
                ):
                    sb_sb = pa.tile([128, NH], F32, name="sb_sb")
                    nc.sync.dma_start(sb_sb[:], scale_b[:])

                    for ci in range(NCH):
                        t0 = ci * TCH
                        tsl = slice(t0, t0 + TCH)
                        cos_sb = pa.tile([128, TCH], F32, name=f"cos_{ci}", tag="cos")
                        nc.sync.dma_start(cos_sb[:], cosT[:, tsl])
                        sin_sb = pa.tile([128, TCH], F32, name=f"sin_{ci}", tag="sin")
                        nc.sync.dma_start(sin_sb[:], sinT[:, tsl])

                        varP = psA.tile([128, TCH], F32, name=f"varP_{ci}", tag="varP")
                        for ct in range(CT):
                            xt = pa3.tile([128, TCH], F32, name=f"xa_{ci}_{ct}", tag="xa")
                            nc.sync.dma_start(xt[:], xT[ct * 128:(ct + 1) * 128, tsl])
                            sq = pa3.tile([128, TCH], BF16, name=f"sq_{ci}_{ct}", tag="sq")
                            nc.vector.tensor_mul(sq[:], xt[:], xt[:])
                            nc.tensor.matmul(
                                varP[:], inv_c[:], sq[:],
                                start=(ct == 0), stop=(ct == CT - 1),
                            )
                        sdv = pa3.tile([128, TCH], F32, name=f"sdv_{ci}", tag="sdv", bufs=2)
                        nc.scalar.activation(sdv[:], varP[:], AF.Sqrt, bias=eps_sb[:, 0:1])
                        rstd = pa3.tile([128, TCH], F32, name=f"rstd_{ci}", tag="rstd", bufs=2)
                        nc.vector.reciprocal(rstd[:], sdv[:])

                        n1 = pa.tile([128, CT, TCH], BF16, name=f"n1_{ci}", tag="n1", bufs=2)
                        for ct in range(CT):
                            xt2 = pa3.tile([128, TCH], F32, name=f"xb_{ci}_{ct}", tag="xb")
                            nc.sync.dma_start(xt2[:], xT[ct * 128:(ct + 1) * 128, tsl])
                            nc.vector.tensor_mul(n1[:, ct, :], xt2[:], rstd[:])

                        # learned score scaling (m-tiles 6-9): mean over head
                        # dim of relu(scale_w @ n1 + b); the (SF/HS)-filled
                        # lhsT of the second matmul both takes the mean and
                        # broadcasts it across all partitions
                        scbc = pa.tile([128, NH, TCH], F32, name=f"scbc_{ci}", tag="scbc")
                        for m in range(NH):
                            swt = pa2.tile([128, CT, 128], BF16, name=f"swt_{ci}_{m}", tag="wA")
                            nc.sync.dma_start(
                                swt[:], wA[6 + m].rearrange("p (ct j) -> p ct j", j=128))
                            scP = psA.tile([128, TCH], F32, name=f"scP_{ci}_{m}", tag="scP", bufs=2)
                            for ct in range(CT):
                                nc.tensor.matmul(
                                    scP[:], swt[:, ct, :], n1[:, ct, :],
                                    start=(ct == 0), stop=(ct == CT - 1),
                                )
                            rel = pa3.tile([128, TCH], BF16, name=f"rel_{ci}_{m}", tag="rel", bufs=2)
                            nc.scalar.activation(rel[:], scP[:], AF.Relu, bias=sb_sb[:, m:m + 1])
                            mscP = psA.tile([128, TCH], F32, name=f"mscP_{ci}_{m}", tag="mscP")
                            nc.tensor.matmul(mscP[:], mean_sf[:], rel[:], start=True, stop=True)
                            nc.scalar.activation(scbc[:, m, :], mscP[:], AF.Copy)

                        # q heads (m 0-3) and k (m 4), with rope; sinT comes
                        # in sign-folded so rot needs no negate
                        for m in range(5):
                            qwt = pa2.tile([128, CT, 128], BF16, name=f"qwt_{ci}_{m}", tag="wA")
                            nc.sync.dma_start(
                                qwt[:], wA[m].rearrange("p (ct j) -> p ct j", j=128))
                            qkP = psA.tile([128, TCH], F32, name=f"qkP_{ci}_{m}", tag="qkP", bufs=3)
                            for ct in range(CT):
                                nc.tensor.matmul(
                                    qkP[:], qwt[:, ct, :], n1[:, ct, :],
                                    start=(ct == 0), stop=(ct == CT - 1),
                                )
                            raw = pa3.tile([128, TCH], F32, name=f"raw_{ci}_{m}", tag="raw", bufs=2)
                            nc.scalar.activation(raw[:], qkP[:], AF.Copy)
                            rot = pa3.tile([128, TCH], F32, name=f"rot_{ci}_{m}", tag="rot", bufs=2)
                            nc.sync.dma_start(rot[0:64, :], raw[64:128, :])
                            nc.sync.dma_start(rot[64:128, :], raw[0:64, :])
                            t1 = pa3.tile([128, TCH], F32, name=f"t1_{ci}_{m}", tag="t1", bufs=2)
                            nc.vector.tensor_mul(t1[:], raw[:], cos_sb[:])
                            t2 = pa3.tile([128, TCH], F32, name=f"t2_{ci}_{m}", tag="t2", bufs=2)
                            nc.vector.tensor_mul(t2[:], rot[:], sin_sb[:])
                            if m < NH:
                                rs = pa3.tile([128, TCH], F32, name=f"rs_{ci}_{m}", tag="rs", bufs=2)
                                nc.vector.tensor_add(rs[:], t1[:], t2[:])
                                nc.vector.tensor_mul(Q_sb[:, m, tsl], rs[:], scbc[:, m, :])
                            else:
                                nc.vector.tensor_add(K_sb[:, tsl], t1[:], t2[:])

                        # v, produced token-major (lhsT = n1 token block,
                        # rhs = v weight tile)
                        vwt = pa2.tile([128, CT, 128], BF16, name=f"vwt_{ci}", tag="wA")
                        nc.sync.dma_start(
                            vwt[:], wA[5].rearrange("p (ct j) -> p ct j", j=128))
                        for tt in range(TCH // 128):
                            vP = psA.tile([128, 128], F32, name=f"vP_{ci}_{tt}", tag="vP")
                            for ct in range(CT):
                                nc.tensor.matmul(
                                    vP[:], n1[:, ct, tt * 128:(tt + 1) * 128],
                                    vwt[:, ct, :],
                                    start=(ct == 0), stop=(ct == CT - 1),
                                )
                            nc.scalar.activation(V_sb[:, ci * 4 + tt, :], vP[:], AF.Copy)

                # ---------- Phase B: attention -----------------------------
                # scores stay unshifted (|s| <= ~10, exp safe in fp32);
                # softmax denominator via ones-matmul partition reduction
                with (
                    tc.tile_pool(name="pb", bufs=3) as pb,
                    tc.tile_pool(name="psB", bufs=1, space="PSUM") as psB,
                ):
                    for h in range(NH):
                        for qc in range(NCH):
                            qsl = slice(qc * 512, qc * 512 + 512)
                            yP = psB.tile([128, 512], F32, name=f"yP_{h}_{qc}", tag="yP", bufs=2)
                            csP = psB.tile([128, 512], F32, name=f"csP_{h}_{qc}", tag="csP", bufs=2)
                            for kt in range(KT):
                                sP = psB.tile([128, 512], F32, name=f"sP_{h}_{qc}_{kt}", tag="sP", bufs=4)
                                nc.tensor.matmul(
                                    sP[:], K_sb[:, kt * 128:(kt + 1) * 128],
                                    Q_sb[:, h, qsl], start=True, stop=True,
                                )
                                E = pb.tile([128, 512], BF16, name=f"E_{h}_{qc}_{kt}", tag="E")
                                nc.scalar.activation(E[:], sP[:], AF.Exp)
                                nc.tensor.matmul(
                                    yP[:], V_sb[:, kt, :], E[:],
                                    start=(kt == 0), stop=(kt == KT - 1),
                                )
                                nc.tensor.matmul(
                                    csP[:], ones128[:], E[:],
                                    start=(kt == 0), stop=(kt == KT - 1),
                                )
                            cs = pb.tile([128, 512], F32, name=f"cs_{h}_{qc}", tag="cs", bufs=2)
                            nc.scalar.activation(cs[:], csP[:], AF.Copy)
                            rb = pb.tile([128, 512], F32, name=f"rb_{h}_{qc}", tag="rb", bufs=2)
                            nc.vector.reciprocal(rb[:], cs[:])
                            nc.vector.tensor_mul(yT_sb[:, h, qsl], yP[:], rb[:])

                # ---------- Phase C: proj partial + AllReduce --------------
                with (
                    tc.tile_pool(name="pc", bufs=1) as pc,
                    tc.tile_pool(name="pc4", bufs=4) as pc4,
                    tc.tile_pool(name="psC", bufs=4, space="PSUM") as psC,
                ):
                    proj_sb = pc.tile([128, NH, C], BF16, name="proj_sb")
                    nc.sync.dma_start(proj_sb[:], proj_w[:])
                    for qc in range(NCH):
                        qsl = slice(qc * 512, qc * 512 + 512)
                        for o in range(CT):
                            hP = psC.tile([128, 512], F32, name=f"hP_{o}_{qc}", tag="hP")
                            for hh in range(NH):
                                nc.tensor.matmul(
                                    hP[:], proj_sb[:, hh, o * 128:(o + 1) * 128],
                                    yT_sb[:, hh, qsl],
                                    start=(hh == 0), stop=(hh == NH - 1),
                                )
                            ho = pc4.tile([128, 512], BF16, name=f"ho_{o}_{qc}", tag="ho")
                            nc.scalar.activation(ho[:], hP[:], AF.Copy)
                            nc.sync.dma_start(cc1_in[qc][o * 128:(o + 1) * 128, :], ho[:])
                        if cc:
                            nc.gpsimd.collective_compute(
                                "AllReduce", ADD,
                                replica_groups=[list(range(N_CORES))],
                                ins=[cc1_in[qc].opt()], outs=[cc1_out[qc].opt()],
                            )
                    if not cc:
                        cc1_out = cc1_in

            # ---------- Phases D+E: rmsnorm2 + MLP, per token-half ---------
            with (
                tc.tile_pool(name="pd", bufs=1) as pd,
                tc.tile_pool(name="pd3", bufs=3) as pd3,
                tc.tile_pool(name="psD", bufs=2, space="PSUM") as psD,
            ):
                for mh in range(2):
                    n2 = pd.tile([128, CT, TM], BF16, name=f"n2_{mh}", tag="n2")
                    with tc.tile_pool(name=f"pdx_{mh}", bufs=1) as pdx:
                        for q2 in range(TM // 512):
                            t0 = mh * TM + q2 * 512
                            tsl = slice(t0, t0 + 512)
                            lsl = slice(q2 * 512, q2 * 512 + 512)
                            x1c = pdx.tile([128, CT, 512], F32, name=f"x1c_{mh}_{q2}", tag="x1c")
                            varP2 = psD.tile([128, 512], F32, name=f"varP2_{mh}_{q2}", tag="varP2", bufs=1)
                            qci = mh * 2 + q2
                            for ct in range(CT):
                                xt = pd3.tile([128, 512], F32, name=f"xd_{mh}_{q2}_{ct}", tag="xd", bufs=2)
                                nc.sync.dma_start(xt[:], xT[ct * 128:(ct + 1) * 128, tsl])
                                ht = pd3.tile([128, 512], BF16, name=f"hd_{mh}_{q2}_{ct}", tag="hd")
                                nc.sync.dma_start(ht[:], cc1_out[qci][ct * 128:(ct + 1) * 128, :])
                                nc.vector.tensor_add(x1c[:, ct, :], xt[:], ht[:])
                                # x1/8 seeds the fp32 ReduceScatter input;
                                # down-proj partials DMA-accumulate on top
                                x18 = pd3.tile([128, 512], F32, name=f"x18_{mh}_{q2}_{ct}", tag="x18", bufs=2)
                                nc.scalar.activation(x18[:], x1c[:, ct, :], AF.Copy, scale=1.0 / N_CORES)
                                nc.sync.dma_start(cc2_in[qci][ct * 128:(ct + 1) * 128, :], x18[:])
                                sq2 = pd3.tile([128, 512], BF16, name=f"sq2_{mh}_{q2}_{ct}", tag="sq2")
                                nc.vector.tensor_mul(sq2[:], x1c[:, ct, :], x1c[:, ct, :])
                                nc.tensor.matmul(
                                    varP2[:], inv_c[:], sq2[:],
                                    start=(ct == 0), stop=(ct == CT - 1),
                                )
                            sdv2 = pd3.tile([128, 512], F32, name=f"sdv2_{mh}_{q2}", tag="sdv2", bufs=1)
                            nc.scalar.activation(sdv2[:], varP2[:], AF.Sqrt, bias=eps_sb[:, 0:1])
                            rstd2 = pd3.tile([128, 512], F32, name=f"rstd2_{mh}_{q2}", tag="rstd2", bufs=1)
                            nc.vector.reciprocal(rstd2[:], sdv2[:])
                            for ct in range(CT):
                                nc.vector.tensor_mul(n2[:, ct, lsl], x1c[:, ct, :], rstd2[:])

                    with tc.tile_pool(name=f"pe_{mh}", bufs=1) as pe:
                        sg = pe.tile([128, FT, TM], BF16, name=f"sg_{mh}")
                        for f in range(FT):
                            gw = pe.tile([128, CT, 128], BF16, name=f"gw_{mh}_{f}", tag="gw", bufs=2)
                            nc.sync.dma_start(gw[:], gate_w[f].rearrange("p (ct j) -> p ct j", j=128))
                            uw = pe.tile([128, CT, 128], BF16, name=f"uw_{mh}_{f}", tag="uw", bufs=2)
                            nc.sync.dma_start(uw[:], up_w[f].rearrange("p (ct j) -> p ct j", j=128))
                            for q2 in range(TM // 512):
                                lsl = slice(q2 * 512, q2 * 512 + 512)
                                gP = psD.tile([128, 512], F32, name=f"gP_{mh}_{f}_{q2}", tag="gP")
                                uP = psD.tile([128, 512], F32, name=f"uP_{mh}_{f}_{q2}", tag="uP")
                                for ct in range(CT):
                                    nc.tensor.matmul(
                                        gP[:], gw[:, ct, :], n2[:, ct, lsl],
                                        start=(ct == 0), stop=(ct == CT - 1),
                                    )
                                for ct in range(CT):
                                    nc.tensor.matmul(
                                        uP[:], uw[:, ct, :], n2[:, ct, lsl],
                                        start=(ct == 0), stop=(ct == CT - 1),
                                    )
                                sig = pd3.tile([128, 512], BF16, name=f"sig_{mh}_{f}_{q2}", tag="sig")
                                nc.scalar.activation(sig[:], gP[:], AF.Sigmoid)
                                m1 = pd3.tile([128, 512], BF16, name=f"m1_{mh}_{f}_{q2}", tag="m1")
                                nc.vector.tensor_mul(m1[:], gP[:], sig[:])
                                nc.vector.tensor_mul(sg[:, f, lsl], m1[:], uP[:])

                        # down-proj with the token-chunk loop outermost so
                        # each chunk's ReduceScatter fires as soon as its
                        # last accumulate lands
                        for q2 in range(TM // 512):
                            qci = mh * 2 + q2
                            t0 = mh * TM + q2 * 512
                            tsl = slice(t0, t0 + 512)
                            lsl = slice(q2 * 512, q2 * 512 + 512)
                            for o in range(CT):
                                dw = pe.tile([128, FT, 128], BF16, name=f"dw_{mh}_{q2}_{o}", tag="dw", bufs=2)
                                nc.sync.dma_start(dw[:], down_w[o].rearrange("p (ft c) -> p ft c", c=128))
                                dP = psD.tile([128, 512], F32, name=f"dP_{mh}_{o}_{q2}", tag="dP", bufs=3)
                                for f in range(FT):
                                    nc.tensor.matmul(
                                        dP[:], dw[:, f, :], sg[:, f, lsl],
                                        start=(f == 0), stop=(f == FT - 1),
                                    )
                                dsb = pd3.tile([128, 512], F32, name=f"dsb_{mh}_{o}_{q2}", tag="dsb", bufs=2)
                                nc.scalar.activation(dsb[:], dP[:], AF.Copy)
                                nc.gpsimd.dma_start(
                                    cc2_in[qci][o * 128:(o + 1) * 128, :], dsb[:], accum_op=ADD,
                                )
                            if cc:
                                nc.gpsimd.collective_compute(
                                    "ReduceScatter", ADD,
                                    replica_groups=[list(range(N_CORES))],
                                    ins=[cc2_in[qci].opt()], outs=[cc2_out[qci].opt()],
                                )

                if not cc:
                    cc2_out = [t[0:512, :] for t in cc2_in]
                for qci in range(NCH):
                    for ot in range(4):
                        ob = pd3.tile([128, TCH], F32, name=f"ob_{qci}_{ot}", tag="ob", bufs=2)
                        nc.sync.dma_start(ob[:], cc2_out[qci][ot * 128:(ot + 1) * 128, :])
                        nc.sync.dma_start(outT[ot * 128:(ot + 1) * 128, qci * TCH:(qci + 1) * TCH], ob[:])

    nc.compile()
    return nc


def _prep_inputs(inputs):
    x = np.asarray(inputs["x"], np.float32)[0]          # [T, C]
    cos = np.asarray(inputs["cos"], np.float32)
    sin = np.asarray(inputs["sin"], np.float32)
    w1 = np.asarray(inputs["norm1_w"], np.float32)
    w2 = np.asarray(inputs["norm2_w"], np.float32)
    attn_w = np.asarray(inputs["attn_w"], np.float32)
    proj_w = np.asarray(inputs["proj_w"], np.float32)
    scale_w = np.asarray(inputs["scale_w"], np.float32)
    scale_b = np.asarray(inputs["scale_b"], np.float32)
    gate_w = np.asarray(inputs["gate_w"], np.float32)
    up_w = np.asarray(inputs["up_w"], np.float32)
    down_w = np.asarray(inputs["down_w"], np.float32)

    xT = np.ascontiguousarray(x.T)                      # [C, T]
    cosT = np.ascontiguousarray(cos.T)                  # [128, T]
    sinTs = sin.T.copy()
    sinTs[0:64] *= -1.0                                 # sign-folded rot half
    sinTs = np.ascontiguousarray(sinTs)

    def lhst_tiles(w, nt):  # [nt*128, C] -> [nt, 128, C] lhsT tile layout
        return np.ascontiguousarray(
            w.reshape(nt, 128, CT, 128).transpose(0, 3, 2, 1).reshape(nt, 128, C)
        ).astype(BF)

    fl = 11008 // N_CORES                               # 1376
    maps = []
    for g in range(N_CORES):
        aw = attn_w[g * 768:(g + 1) * 768] * w1[None, :]
        sw = scale_w[g * 512:(g + 1) * 512] * w1[None, :]
        wa_dev = lhst_tiles(np.concatenate([aw, sw], axis=0), 10)
        sb = np.ascontiguousarray(scale_b[g * 512:(g + 1) * 512].reshape(NH, 128).T)
        pw = proj_w[:, g * 512:(g + 1) * 512].T         # [512, C]
        pw_dev = np.ascontiguousarray(
            pw.reshape(NH, 128, C).transpose(1, 0, 2)
        ).astype(BF)

        gsh = np.zeros((FT * 128, C), np.float32)
        gsh[:fl] = gate_w[g * fl:(g + 1) * fl] * w2[None, :]
        ush = np.zeros((FT * 128, C), np.float32)
        ush[:fl] = up_w[g * fl:(g + 1) * fl] * w2[None, :]
        dsh = np.zeros((FT * 128, C), np.float32)
        dsh[:fl] = down_w[:, g * fl:(g + 1) * fl].T

        g_dev = lhst_tiles(gsh, FT)
        u_dev = lhst_tiles(ush, FT)
        # down lhsT tiles: [ot, p(f), ft*128+c2] with value W[ft*128+p, ot*128+c2]
        d_dev = np.ascontiguousarray(
            dsh.reshape(FT, 128, CT, 128).transpose(2, 1, 0, 3).reshape(CT, 128, FT * 128)
        ).astype(BF)

        maps.append({
            "xT": xT,
            "cosT": cosT,
            "sinT": sinTs,
            "wA": wa_dev,
            "scale_b": sb,
            "proj_w": pw_dev,
            "gate_w": g_dev,
            "up_w": u_dev,
            "down_w": d_dev,
        })
    return maps


def _run(inputs, **kw):
    if "nc" not in _CACHE:
        _CACHE["nc"] = _build()
    nc = _CACHE["nc"]
    maps = _prep_inputs(inputs)
    res = run_bass_kernel_spmd(nc, maps, core_ids=list(range(N_CORES)), **kw)
    full = np.concatenate([res.results[g]["outT"] for g in range(N_CORES)], axis=0)
    out = np.ascontiguousarray(full.T)[None].astype(np.float32)
    return out, res


def kernel(**inputs):
    out, _ = _run(inputs)
    return out


def kernel_traced(**inputs):
    out, res = _run(inputs, trace=True)
    return out, res



# revision 7
# speedup vs baseline: 19.5788x; 19.5788x over previous
"""Variant 3: K/V AllGather + token-sharded everything else, on 8 cores.

Each core computes rmsnorm1 over all T tokens and K/V for ITS OWN GQA group
(tensor-parallel, no redundancy), publishes them with one small AllGather
(1 MB in / 8 MB out, bf16) that overlaps with the Q/score-scaling compute for
its own 256-token slice, then runs attention (all 32 heads), proj, rmsnorm2
and the FULL-ffn SwiGLU MLP for its own tokens only.  The single collective
is tiny and hidden behind compute; each core writes its own token slice of
the output with no further sync.
"""

import numpy as np
import ml_dtypes

import concourse.mybir as mybir
import concourse.tile as tile
from concourse import bacc
from concourse.bass_utils import run_bass_kernel_spmd

BF = ml_dtypes.bfloat16
F32 = mybir.dt.float32
BF16 = mybir.dt.bfloat16
AF = mybir.ActivationFunctionType

N_CORES = 8
T = 2048
C = 4096
HS = 128
NHF = 32          # full q heads
NG = 8            # kv groups
CT = C // 128     # 32 c-tiles
TCH = 512         # token chunk, phase A full-T pass
NCH = T // TCH
KT = T // 128     # 16 key tiles
FT = 86           # full ffn tiles (86*128 = 11008)
TO = T // N_CORES  # 256 own tokens
EPS = 1e-5
SF = 1.0 / float(np.sqrt(HS))

_CACHE = {}


def _build(cc=True):
    nc = bacc.Bacc(None, target_bir_lowering=False, num_devices=N_CORES)

    xT = nc.dram_tensor("xT", [C, T], F32, kind="ExternalInput")
    xT_own = nc.dram_tensor("xT_own", [C, TO], F32, kind="ExternalInput")
    cosT = nc.dram_tensor("cosT", [128, T], F32, kind="ExternalInput")
    sinT = nc.dram_tensor("sinT", [128, T], F32, kind="ExternalInput")
    cos_own = nc.dram_tensor("cos_own", [128, TO], F32, kind="ExternalInput")
    sin_own = nc.dram_tensor("sin_own", [128, TO], F32, kind="ExternalInput")
    # K then V tile for this core's own group, w1-folded
    wKV = nc.dram_tensor("wKV", [2, 128, C], BF16, kind="ExternalInput")
    # all 32 q head tiles then all 32 scale tiles, w1-folded
    wQS = nc.dram_tensor("wQS", [2 * NHF, 128, C], BF16, kind="ExternalInput")
    scale_b = nc.dram_tensor("scale_b", [128, NHF], F32, kind="ExternalInput")
    projT = nc.dram_tensor("projT", [128, NHF, C], BF16, kind="ExternalInput")
    gate_w = nc.dram_tensor("gate_w", [FT, 128, C], BF16, kind="ExternalInput")
    up_w = nc.dram_tensor("up_w", [FT, 128, C], BF16, kind="ExternalInput")
    down_w = nc.dram_tensor("down_w", [CT, 128, FT * 128], BF16, kind="ExternalInput")
    outT = nc.dram_tensor("outT", [C, TO], F32, kind="ExternalOutput")

    with tile.TileContext(nc) as tc:
        with (
            tc.tile_pool(name="persist", bufs=1) as pp,
            tc.tile_pool(name="dram", bufs=1, space="DRAM") as dram,
        ):
            inv_c = pp.tile([128, 128], BF16, name="inv_c")
            nc.vector.memset(inv_c[:], 1.0 / C)
            mean_sf = pp.tile([128, 128], BF16, name="mean_sf")
            nc.vector.memset(mean_sf[:], SF / HS)
            ones128 = pp.tile([128, 128], BF16, name="ones128")
            nc.vector.memset(ones128[:], 1.0)
            eps_sb = pp.tile([128, 1], F32, name="eps_sb")
            nc.vector.memset(eps_sb[:], EPS)

            kv_in = dram.tile([2, 128, T], BF16, name="kv_in")
            kv_out = dram.tile([2 * NG, 128, T], BF16, name="kv_out",
                               addr_space="Shared")

            with tc.tile_pool(name="pqkv", bufs=1) as pqkv:
                Q_sb = pqkv.tile([128, NHF, TO], BF16, name="Q_sb")
                K_sb = pqkv.tile([128, NG, T], BF16, name="K_sb")
                V_sb = pqkv.tile([128, NG, KT, 128], BF16, name="V_sb")
                yT_sb = pqkv.tile([128, NHF, TO], BF16, name="yT_sb")

                # ---- Phase A1: full-T rmsnorm1 -> K (rope) and V ----------
                with (
                    tc.tile_pool(name="pa", bufs=1) as pa,
                    tc.tile_pool(name="pa2", bufs=2) as pa2,
                    tc.tile_pool(name="pa3", bufs=3) as pa3,
                    tc.tile_pool(name="psA", bufs=1, space="PSUM") as psA,
                ):
                    # variance pass for all chunks first: keeps the PE fed
                    # with back-to-back var matmuls instead of stalling on the
                    # sqrt->recip->n1 chain between chunks
                    rstd_t = {}
                    for ci in range(NCH):
                        t0 = ci * TCH
                        tsl = slice(t0, t0 + TCH)
                        varP = psA.tile([128, TCH], F32, name=f"varP_{ci}", tag="varP", bufs=2)
                        for ct in range(CT):
                            xt = pa3.tile([128, TCH], F32, name=f"xa_{ci}_{ct}", tag="xa")
                            nc.sync.dma_start(xt[:], xT[ct * 128:(ct + 1) * 128, tsl])
                            sq = pa3.tile([128, TCH], BF16, name=f"sq_{ci}_{ct}", tag="sq")
                            nc.vector.tensor_mul(sq[:], xt[:], xt[:])
                            nc.tensor.matmul(
                                varP[:], inv_c[:], sq[:],
                                start=(ct == 0), stop=(ct == CT - 1),
                            )
                        sdv = pa3.tile([128, TCH], F32, name=f"sdv_{ci}", tag="sdv", bufs=2)
                        nc.scalar.activation(sdv[:], varP[:], AF.Sqrt, bias=eps_sb[:, 0:1])
                        rstd = pa.tile([128, TCH], F32, name=f"rstd_{ci}", tag="rstd", bufs=NCH)
                        nc.vector.reciprocal(rstd[:], sdv[:])
                        rstd_t[ci] = rstd

                    for ci in range(NCH):
                        t0 = ci * TCH
                        tsl = slice(t0, t0 + TCH)
                        cos_sb = pa.tile([128, TCH], F32, name=f"cos_{ci}", tag="cos", bufs=2)
                        nc.sync.dma_start(cos_sb[:], cosT[:, tsl])
                        sin_sb = pa.tile([128, TCH], F32, name=f"sin_{ci}", tag="sin", bufs=2)
                        nc.sync.dma_start(sin_sb[:], sinT[:, tsl])
                        rstd = rstd_t[ci]

                        n1 = pa.tile([128, CT, TCH], BF16, name=f"n1_{ci}", tag="n1", bufs=1)
                        for ct in range(CT):
                            xt2 = pa3.tile([128, TCH], F32, name=f"xb_{ci}_{ct}", tag="xb")
                            nc.sync.dma_start(xt2[:], xT[ct * 128:(ct + 1) * 128, tsl])
                            nc.vector.tensor_mul(n1[:, ct, :], xt2[:], rstd[:])

                        # K for own group, with rope (sign-folded sin)
                        kwt = pa2.tile([128, CT, 128], BF16, name=f"kwt_{ci}", tag="wKV")
                        nc.sync.dma_start(
                            kwt[:], wKV[0].rearrange("p (ct j) -> p ct j", j=128))
                        kP = psA.tile([128, TCH], F32, name=f"kP_{ci}", tag="kP", bufs=3)
                        for ct in range(CT):
                            nc.tensor.matmul(
                                kP[:], kwt[:, ct, :], n1[:, ct, :],
                                start=(ct == 0), stop=(ct == CT - 1),
                            )
                        raw = pa3.tile([128, TCH], F32, name=f"raw_{ci}", tag="raw", bufs=2)
                        nc.scalar.activation(raw[:], kP[:], AF.Copy)
                        rot = pa3.tile([128, TCH], F32, name=f"rot_{ci}", tag="rot", bufs=2)
                        nc.sync.dma_start(rot[0:64, :], raw[64:128, :])
                        nc.sync.dma_start(rot[64:128, :], raw[0:64, :])
                        t1 = pa3.tile([128, TCH], F32, name=f"t1_{ci}", tag="t1", bufs=2)
                        nc.vector.tensor_mul(t1[:], raw[:], cos_sb[:])
                        t2 = pa3.tile([128, TCH], F32, name=f"t2_{ci}", tag="t2", bufs=2)
                        nc.vector.tensor_mul(t2[:], rot[:], sin_sb[:])
                        ko = pa3.tile([128, TCH], BF16, name=f"ko_{ci}", tag="ko", bufs=2)
                        nc.vector.tensor_add(ko[:], t1[:], t2[:])
                        nc.sync.dma_start(kv_in[0, :, tsl], ko[:])

                        # V for own group, token-major
                        vwt = pa2.tile([128, CT, 128], BF16, name=f"vwt_{ci}", tag="wKV")
                        nc.sync.dma_start(
                            vwt[:], wKV[1].rearrange("p (ct j) -> p ct j", j=128))
                        for tt in range(TCH // 128):
                            vP = psA.tile([128, 128], F32, name=f"vP_{ci}_{tt}", tag="vP", bufs=2)
                            for ct in range(CT):
                                nc.tensor.matmul(
                                    vP[:], n1[:, ct, tt * 128:(tt + 1) * 128],
                                    vwt[:, ct, :],
                                    start=(ct == 0), stop=(ct == CT - 1),
                                )
                            vo = pa3.tile([128, 128], BF16, name=f"vo_{ci}_{tt}", tag="vo", bufs=2)
                            nc.scalar.activation(vo[:], vP[:], AF.Copy)
                            nc.sync.dma_start(
                                kv_in[1, :, (ci * 4 + tt) * 128:(ci * 4 + tt + 1) * 128],
                                vo[:])

                    if cc:
                        nc.gpsimd.collective_compute(
                            "AllGather", mybir.AluOpType.bypass,
                            replica_groups=[list(range(N_CORES))],
                            ins=[kv_in.opt()], outs=[kv_out.opt()],
                        )

                # ---- Phase A2: own-token rmsnorm1 -> Q (rope) + scaling ---
                with (
                    tc.tile_pool(name="pq", bufs=1) as pq,
                    tc.tile_pool(name="pq2", bufs=2) as pq2,
                    tc.tile_pool(name="pq3", bufs=3) as pq3,
                    tc.tile_pool(name="psQ", bufs=1, space="PSUM") as psQ,
                ):
                    scbc = pq.tile([128, NHF, TO], BF16, name="scbc")
                    sb_sb = pq.tile([128, NHF], F32, name="sb_sb")
                    nc.sync.dma_start(sb_sb[:], scale_b[:])
                    co_sb = pq.tile([128, TO], F32, name="co_sb")
                    nc.sync.dma_start(co_sb[:], cos_own[:])
                    so_sb = pq.tile([128, TO], F32, name="so_sb")
                    nc.sync.dma_start(so_sb[:], sin_own[:])

                    varQ = psQ.tile([128, TO], F32, name="varQ")
                    for ct in range(CT):
                        xt = pq3.tile([128, TO], F32, name=f"xq_{ct}", tag="xq")
                        nc.sync.dma_start(xt[:], xT_own[ct * 128:(ct + 1) * 128, :])
                        sq = pq3.tile([128, TO], BF16, name=f"sqq_{ct}", tag="sqq")
                        nc.vector.tensor_mul(sq[:], xt[:], xt[:])
                        nc.tensor.matmul(
                            varQ[:], inv_c[:], sq[:],
                            start=(ct == 0), stop=(ct == CT - 1),
                        )
                    sdvq = pq3.tile([128, TO], F32, name="sdvq", bufs=1)
                    nc.scalar.activation(sdvq[:], varQ[:], AF.Sqrt, bias=eps_sb[:, 0:1])
                    rstdq = pq3.tile([128, TO], F32, name="rstdq", bufs=1)
                    nc.vector.reciprocal(rstdq[:], sdvq[:])
                    n1o = pq.tile([128, CT, TO], BF16, name="n1o")
                    for ct in range(CT):
                        xt2 = pq3.tile([128, TO], F32, name=f"xq2_{ct}", tag="xq2")
                        nc.sync.dma_start(xt2[:], xT_own[ct * 128:(ct + 1) * 128, :])
                        nc.vector.tensor_mul(n1o[:, ct, :], xt2[:], rstdq[:])

                    # score scaling for all 32 heads on own tokens
                    for m in range(NHF):
                        swt = pq2.tile([128, CT, 128], BF16, name=f"swt_{m}", tag="wQS")
                        nc.sync.dma_start(
                            swt[:], wQS[NHF + m].rearrange("p (ct j) -> p ct j", j=128))
                        scP = psQ.tile([128, TO], F32, name=f"scP_{m}", tag="scP", bufs=2)
                        for ct in range(CT):
                            nc.tensor.matmul(
                                scP[:], swt[:, ct, :], n1o[:, ct, :],
                                start=(ct == 0), stop=(ct == CT - 1),
                            )
                        rel = pq3.tile([128, TO], BF16, name=f"rel_{m}", tag="rel", bufs=2)
                        nc.scalar.activation(rel[:], scP[:], AF.Relu, bias=sb_sb[:, m:m + 1])
                        mscP = psQ.tile([128, TO], F32, name=f"mscP_{m}", tag="mscP", bufs=2)
                        nc.tensor.matmul(mscP[:], mean_sf[:], rel[:], start=True, stop=True)
                        nc.scalar.activation(scbc[:, m, :], mscP[:], AF.Copy)

                    # Q for all 32 heads on own tokens, rope + scale folded in
                    for m in range(NHF):
                        qwt = pq2.tile([128, CT, 128], BF16, name=f"qwt_{m}", tag="wQS")
                        nc.sync.dma_start(
                            qwt[:], wQS[m].rearrange("p (ct j) -> p ct j", j=128))
                        qP = psQ.tile([128, TO], F32, name=f"qP_{m}", tag="qP", bufs=3)
                        for ct in range(CT):
                            nc.tensor.matmul(
                                qP[:], qwt[:, ct, :], n1o[:, ct, :],
                                start=(ct == 0), stop=(ct == CT - 1),
                            )
                        raw = pq3.tile([128, TO], F32, name=f"rawq_{m}", tag="rawq", bufs=2)
                        nc.scalar.activation(raw[:], qP[:], AF.Copy)
                        rot = pq3.tile([128, TO], F32, name=f"rotq_{m}", tag="rotq", bufs=2)
                        nc.sync.dma_start(rot[0:64, :], raw[64:128, :])
                        nc.sync.dma_start(rot[64:128, :], raw[0:64, :])
                        t1 = pq3.tile([128, TO], F32, name=f"t1q_{m}", tag="t1q", bufs=2)
                        nc.vector.tensor_mul(t1[:], raw[:], co_sb[:])
                        t2 = pq3.tile([128, TO], F32, name=f"t2q_{m}", tag="t2q", bufs=2)
                        nc.vector.tensor_mul(t2[:], rot[:], so_sb[:])
                        rs = pq3.tile([128, TO], F32, name=f"rsq_{m}", tag="rsq", bufs=2)
                        nc.vector.tensor_add(rs[:], t1[:], t2[:])
                        nc.vector.tensor_mul(Q_sb[:, m, :], rs[:], scbc[:, m, :])

                # ---- Phase B: attention, 32 heads x own 256 queries -------
                with (
                    tc.tile_pool(name="pb", bufs=3) as pb,
                    tc.tile_pool(name="psB", bufs=1, space="PSUM") as psB,
                ):
                    for g in range(NG):
                        nc.sync.dma_start(K_sb[:, g, :], kv_out[2 * g])
                        nc.sync.dma_start(
                            V_sb[:, g, :, :],
                            kv_out[2 * g + 1].rearrange("p (kt j) -> p kt j", j=128))
                    for h in range(NHF):
                        g = h // 4
                        yP = psB.tile([128, TO], F32, name=f"yP_{h}", tag="yP", bufs=2)
                        csP = psB.tile([128, TO], F32, name=f"csP_{h}", tag="csP", bufs=2)
                        for kt in range(KT):
                            sP = psB.tile([128, TO], F32, name=f"sP_{h}_{kt}", tag="sP", bufs=4)
                            nc.tensor.matmul(
                                sP[:], K_sb[:, g, kt * 128:(kt + 1) * 128],
                                Q_sb[:, h, :], start=True, stop=True,
                            )
                            E = pb.tile([128, TO], BF16, name=f"E_{h}_{kt}", tag="E")
                            nc.scalar.activation(E[:], sP[:], AF.Exp)
                            nc.tensor.matmul(
                                yP[:], V_sb[:, g, kt, :], E[:],
                                start=(kt == 0), stop=(kt == KT - 1),
                            )
                            nc.tensor.matmul(
                                csP[:], ones128[:], E[:],
                                start=(kt == 0), stop=(kt == KT - 1),
                            )
                        cs = pb.tile([128, TO], F32, name=f"cs_{h}", tag="cs", bufs=2)
                        nc.scalar.activation(cs[:], csP[:], AF.Copy)
                        rb = pb.tile([128, TO], F32, name=f"rb_{h}", tag="rb", bufs=2)
                        nc.vector.reciprocal(rb[:], cs[:])
                        nc.vector.tensor_mul(yT_sb[:, h, :], yP[:], rb[:])

            # ---- Phase C+D: proj (feature-major) + x1 + rmsnorm2 ----------
            with (
                tc.tile_pool(name="pd", bufs=1) as pd,
                tc.tile_pool(name="pd2", bufs=2) as pd2,
                tc.tile_pool(name="pd3", bufs=3) as pd3,
            ):
                x1T = pd.tile([128, CT, TO], F32, name="x1T")
                n2T = pd.tile([128, CT, TO], BF16, name="n2T")
                with tc.tile_pool(name="psC", bufs=1, space="PSUM") as psC:
                    varP2 = psC.tile([128, TO], F32, name="varP2", bufs=1)
                    for ct in range(CT):
                        pwt = pd2.tile([128, NHF, 128], BF16, name=f"pwt_{ct}", tag="pwt")
                        nc.sync.dma_start(pwt[:], projT[:, :, ct * 128:(ct + 1) * 128])
                        hP = psC.tile([128, TO], F32, name=f"hP_{ct}", tag="hP", bufs=3)
                        for hh in range(NHF):
                            nc.tensor.matmul(
                                hP[:], pwt[:, hh, :], yT_sb[:, hh, :],
                                start=(hh == 0), stop=(hh == NHF - 1),
                            )
                        xt = pd3.tile([128, TO], F32, name=f"xd_{ct}", tag="xd")
                        nc.sync.dma_start(xt[:], xT_own[ct * 128:(ct + 1) * 128, :])
                        nc.vector.tensor_add(x1T[:, ct, :], xt[:], hP[:])
                        sq2 = pd3.tile([128, TO], BF16, name=f"sq2_{ct}", tag="sq2")
                        nc.vector.tensor_mul(sq2[:], x1T[:, ct, :], x1T[:, ct, :])
                        nc.tensor.matmul(
                            varP2[:], inv_c[:], sq2[:],
                            start=(ct == 0), stop=(ct == CT - 1),
                        )
                    sdv2 = pd3.tile([128, TO], F32, name="sdv2", bufs=1)
                    nc.scalar.activation(sdv2[:], varP2[:], AF.Sqrt, bias=eps_sb[:, 0:1])
                    rstd2 = pd3.tile([128, TO], F32, name="rstd2", bufs=1)
                    nc.vector.reciprocal(rstd2[:], sdv2[:])
                    for ct in range(CT):
                        nc.vector.tensor_mul(n2T[:, ct, :], x1T[:, ct, :], rstd2[:])

                # ---- Phase E: full-ffn SwiGLU MLP for own tokens ----------
                with (
                    tc.tile_pool(name="pe", bufs=1) as pe,
                    tc.tile_pool(name="psD", bufs=2, space="PSUM") as psD,
                ):
                    sg = pe.tile([128, FT, TO], BF16, name="sg")
                    for f in range(FT):
                        gw = pe.tile([128, CT, 128], BF16, name=f"gw_{f}", tag="gw", bufs=2)
                        nc.sync.dma_start(gw[:], gate_w[f].rearrange("p (ct j) -> p ct j", j=128))
                        uw = pe.tile([128, CT, 128], BF16, name=f"uw_{f}", tag="uw", bufs=2)
                        nc.sync.dma_start(uw[:], up_w[f].rearrange("p (ct j) -> p ct j", j=128))
                        gP = psD.tile([128, TO], F32, name=f"gP_{f}", tag="gP", bufs=2)
                        uP = psD.tile([128, TO], F32, name=f"uP_{f}", tag="uP", bufs=2)
                        for ct in range(CT):
                            nc.tensor.matmul(
                                gP[:], gw[:, ct, :], n2T[:, ct, :],
                                start=(ct == 0), stop=(ct == CT - 1),
                            )
                        for ct in range(CT):
                            nc.tensor.matmul(
                                uP[:], uw[:, ct, :], n2T[:, ct, :],
                                start=(ct == 0), stop=(ct == CT - 1),
                            )
                        sig = pd3.tile([128, TO], BF16, name=f"sig_{f}", tag="sig")
                        nc.scalar.activation(sig[:], gP[:], AF.Sigmoid)
                        m1 = pd3.tile([128, TO], BF16, name=f"m1_{f}", tag="m1")
                        nc.vector.tensor_mul(m1[:], gP[:], sig[:])
                        nc.vector.tensor_mul(sg[:, f, :], m1[:], uP[:])

                    for o in range(CT):
                        dw = pe.tile([128, FT, 128], BF16, name=f"dw_{o}", tag="dw", bufs=2)
                        nc.sync.dma_start(dw[:], down_w[o].rearrange("p (ft c) -> p ft c", c=128))
                        dP = psD.tile([128, TO], F32, name=f"dP_{o}", tag="dP", bufs=3)
                        for f in range(FT):
                            nc.tensor.matmul(
                                dP[:], dw[:, f, :], sg[:, f, :],
                                start=(f == 0), stop=(f == FT - 1),
                            )
                        ob = pd3.tile([128, TO], F32, name=f"ob_{o}", tag="ob", bufs=2)
                        nc.vector.tensor_add(ob[:], x1T[:, o, :], dP[:])
                        nc.sync.dma_start(outT[o * 128:(o + 1) * 128, :], ob[:])

    nc.compile()
    return nc


def _prep_inputs(inputs):
    x = np.asarray(inputs["x"], np.float32)[0]          # [T, C]
    cos = np.asarray(inputs["cos"], np.float32)
    sin = np.asarray(inputs["sin"], np.float32)
    w1 = np.asarray(inputs["norm1_w"], np.float32)
    w2 = np.asarray(inputs["norm2_w"], np.float32)
    attn_w = np.asarray(inputs["attn_w"], np.float32)
    proj_w = np.asarray(inputs["proj_w"], np.float32)
    scale_w = np.asarray(inputs["scale_w"], np.float32)
    scale_b = np.asarray(inputs["scale_b"], np.float32)
    gate_w = np.asarray(inputs["gate_w"], np.float32)
    up_w = np.asarray(inputs["up_w"], np.float32)
    down_w = np.asarray(inputs["down_w"], np.float32)

    xT = np.ascontiguousarray(x.T)                      # [C, T]
    cosT = np.ascontiguousarray(cos.T)                  # [128, T]
    sinTs = sin.T.copy()
    sinTs[0:64] *= -1.0                                 # sign-folded rot half
    sinTs = np.ascontiguousarray(sinTs)

    def lhst_tiles(w, nt):  # [nt*128, C] -> [nt, 128, C] lhsT tile layout
        return np.ascontiguousarray(
            w.reshape(nt, 128, CT, 128).transpose(0, 3, 2, 1).reshape(nt, 128, C)
        ).astype(BF)

    # all q head rows (head h = group h//4, sub q h%4) then all scale rows
    q_rows = np.concatenate(
        [attn_w[(h // 4) * 768 + (h % 4) * 128: (h // 4) * 768 + (h % 4) * 128 + 128]
         for h in range(NHF)], axis=0)
    wqs_dev = lhst_tiles(
        np.concatenate([q_rows, scale_w], axis=0) * w1[None, :], 2 * NHF)

    sb_dev = np.ascontiguousarray(scale_b.reshape(NHF, 128).T)
    # projT[d, h, c] = proj_w[c, h*128+d]
    pw_dev = np.ascontiguousarray(
        proj_w.reshape(C, NHF, 128).transpose(2, 1, 0)
    ).astype(BF)

    g_dev = lhst_tiles(gate_w * w2[None, :], FT)
    u_dev = lhst_tiles(up_w * w2[None, :], FT)
    d_dev = np.ascontiguousarray(
        down_w.reshape(CT, 128, FT, 128).transpose(0, 3, 2, 1).reshape(CT, 128, FT * 128)
    ).astype(BF)

    maps = []
    for g in range(N_CORES):
        osl = slice(g * TO, (g + 1) * TO)
        kv_rows = np.concatenate(
            [attn_w[g * 768 + 512: g * 768 + 640],
             attn_w[g * 768 + 640: g * 768 + 768]], axis=0)
        wkv_dev = lhst_tiles(kv_rows * w1[None, :], 2)
        maps.append({
            "xT": xT,
            "xT_own": np.ascontiguousarray(xT[:, osl]),
            "cosT": cosT,
            "sinT": sinTs,
            "cos_own": np.ascontiguousarray(cosT[:, osl]),
            "sin_own": np.ascontiguousarray(sinTs[:, osl]),
            "wKV": wkv_dev,
            "wQS": wqs_dev,
            "scale_b": sb_dev,
            "projT": pw_dev,
            "gate_w": g_dev,
            "up_w": u_dev,
            "down_w": d_dev,
        })
    return maps


def _run(inputs, **kw):
    if "nc" not in _CACHE:
        _CACHE["nc"] = _build()
    nc = _CACHE["nc"]
    maps = _prep_inputs(inputs)
    res = run_bass_kernel_spmd(nc, maps, core_ids=list(range(N_CORES)), **kw)
    full = np.concatenate([res.results[g]["outT"] for g in range(N_CORES)], axis=1)
    out = np.ascontiguousarray(full.T)[None].astype(np.float32)
    return out, res


def kernel(**inputs):
    out, _ = _run(inputs)
    return out


def kernel_traced(**inputs):
    out, res = _run(inputs, trace=True)
    return out, res
